# revision 1
# baseline (speedup 1.0000x reference)
"""Trainium2 Bass kernel for nn_MEGANCore (GATv2-style message-passing GNN).

Key insight 1: in the reference, _gatv2 gathers x_j = xp[col] and segment-sums
x_j * alpha by col; softmax weights alpha sum to 1 within each segment (and
self-loops guarantee non-empty segments), so the aggregation is exactly
xp = h @ W: the edges never matter.  The network collapses to a per-node
linear chain + layernorms + pooling + MLP.

Key insight 2 (folding): with ln_bias == 0 (asserted), each layer is
    h_{l+1} = rstd_l * (h_l @ B_l),   B_l = diag(scale_{l-1}) (I + (W0+W1)/2) C
with C = I - 11^T/64 the centering matrix and rstd a per-node scalar.
Per-node scalars commute through the chain; dropping the LN eps=1e-5 inside
the chain (verified 3e-6 absmax-relative on the final output) the scalars
all cancel except a final c4 = 1/sqrt(mean((x @ B*)^2)) with
B* = B0@B1@B2@B3 precomputed on host.  The device computes only:

    h~ = x @ B*                    (one 64x64 matmul per 128-node block)
    c4 = rsqrt(mean(h~^2, feat))   (per node)
    g  = (Mpool * c4)^T @ h~       (pooling, 8 graphs/core)
    out = relu(g@W1'+b1)@W2+b2     (W1' = diag(ln_scale[3]) @ W1)

Sharding: batch is sorted; 64 graphs -> 8 graphs per core, contiguous node
ranges padded to NPAD.  Host prep is pure data layout (transpose/pad/
one-hot/weight folding).  Matmuls run as float32r (full fp32 storage,
fast PE mode; measured 2.3e-4 absmax-relative error), fp32 statistics.
"""

import numpy as np

HID = 64
NCORES = 8
GPC = 8                 # graphs per core
NBLK = 52               # 128-node blocks per core
NPAD = NBLK * 128       # 6656 padded nodes per core
QB = 13                 # blocks per psum quarter
EPS_SQ = 1e-9           # guards rsqrt on zero-padded nodes

_prog = None


def _build_program():
    import concourse.tile as tile
    from concourse import bacc, mybir
    from contextlib import ExitStack

    f32 = mybir.dt.float32
    f32r = mybir.dt.float32r
    bf16 = mybir.dt.bfloat16

    nc = bacc.Bacc(
        "TRN2", target_bir_lowering=False, debug=False, num_devices=NCORES
    )
    xT = nc.dram_tensor("xT", [64, NPAD], f32r, kind="ExternalInput").ap()
    Bs = nc.dram_tensor("Bs", [64, 64], f32r, kind="ExternalInput").ap()
    Mp = nc.dram_tensor("Mp", [128, NBLK * GPC], f32, kind="ExternalInput").ap()
    W1 = nc.dram_tensor("W1", [64, 32], f32r, kind="ExternalInput").ap()
    b1 = nc.dram_tensor("b1", [32, 1], f32, kind="ExternalInput").ap()
    W2 = nc.dram_tensor("W2", [32, 1], f32r, kind="ExternalInput").ap()
    b2 = nc.dram_tensor("b2", [1, 1], f32, kind="ExternalInput").ap()
    ey = nc.dram_tensor("ey", [8, 8], f32, kind="ExternalInput").ap()
    out = nc.dram_tensor("out", [1, GPC], f32, kind="ExternalOutput").ap()

    with tile.TileContext(nc) as tc:
        with ExitStack() as ctx:
            _body(ctx, tc, nc, mybir, xT, Bs, Mp, W1, b1, W2, b2, ey, out)
    nc.compile()
    return nc


def _body(ctx, tc, nc, mybir, xT, Bs, Mp, W1, b1, W2, b2, ey, out):
    f32 = mybir.dt.float32
    f32r = mybir.dt.float32r
    bf16 = mybir.dt.bfloat16
    AF = mybir.ActivationFunctionType
    AX = mybir.AxisListType
    ALU = mybir.AluOpType

    const = ctx.enter_context(tc.tile_pool(name="const", bufs=1))
    spool = ctx.enter_context(tc.tile_pool(name="scr", bufs=1))
    xpool = ctx.enter_context(tc.tile_pool(name="xp", bufs=1))
    l3p = ctx.enter_context(tc.tile_pool(name="l3p", bufs=2, space="PSUM"))
    gps = ctx.enter_context(tc.tile_pool(name="gps", bufs=1, space="PSUM"))

    Bsb = const.tile([64, 64], f32r, tag="Bsb")
    nc.sync.dma_start(Bsb[:], Bs)
    Mpsb = const.tile([128, NBLK * GPC], f32, tag="Mpsb")
    nc.sync.dma_start(Mpsb[:], Mp)
    W1sb = const.tile([64, 32], f32r, tag="W1sb")
    nc.sync.dma_start(W1sb[:], W1)
    b1sb = const.tile([32, 1], f32, tag="b1sb")
    nc.sync.dma_start(b1sb[:], b1)
    W2sb = const.tile([32, 1], f32r, tag="W2sb")
    nc.sync.dma_start(W2sb[:], W2)
    b2sb = const.tile([1, 1], f32, tag="b2sb")
    nc.sync.dma_start(b2sb[:], b2)
    eysb = const.tile([8, 8], f32, tag="eysb")
    nc.sync.dma_start(eysb[:], ey)
    epsb = const.tile([128, 1], f32, tag="epsb")
    nc.vector.memset(epsb[:], EPS_SQ)

    # ---- load x (feat-major, host-transposed), per-quarter chunks ----
    xsb = xpool.tile([64, NPAD], f32r, tag="xsb")
    for q in range(4):
        nc.sync.dma_start(
            xsb[:, q * QB * 128:(q + 1) * QB * 128],
            xT[:, q * QB * 128:(q + 1) * QB * 128],
        )

    # ---- h~ = x @ B* per 128-node block (node-major out), stats, evict ----
    y3 = spool.tile([128, NBLK * 64], f32r, tag="y3")
    sq = spool.tile([128, NBLK * 64], f32, tag="sq")
    msq = spool.tile([128, NBLK], f32, tag="msq")
    for q in range(4):
        ps = l3p.tile([128, QB * 64], f32, tag="l3")
        for i in range(QB):
            t = q * QB + i
            nc.tensor.matmul(
                ps[:, i * 64:(i + 1) * 64],
                xsb[:, t * 128:(t + 1) * 128],
                Bsb[:],
                start=True, stop=True,
            )
        half = QB * 64 // 2  # split eviction DVE/ACT
        q0 = q * QB * 64
        nc.vector.tensor_copy(y3[:, q0:q0 + half], ps[:, :half])
        nc.scalar.copy(y3[:, q0 + half:q0 + QB * 64], ps[:, half:])
        nc.scalar.square(sq[:, q0:q0 + QB * 64], ps[:])
        nc.vector.tensor_reduce(
            msq[:, q * QB:(q + 1) * QB],
            sq[:, q0:q0 + QB * 64].rearrange("p (b f) -> p b f", f=64),
            axis=AX.X, op=ALU.add,
        )

    # ---- c4 = 1/sqrt(msq/64 + eps), folded into pooling weights ----
    c4a = spool.tile([128, NBLK], f32, tag="c4a")
    nc.scalar.activation(c4a[:], msq[:], AF.Sqrt, bias=epsb[:], scale=1.0 / 64)
    c4 = spool.tile([128, NBLK], f32, tag="c4")
    nc.vector.reciprocal(c4[:], c4a[:])

    mp2 = spool.tile([128, NBLK * GPC], f32r, tag="mp2")
    for t in range(NBLK):
        nc.vector.tensor_scalar_mul(
            mp2[:, t * GPC:(t + 1) * GPC],
            Mpsb[:, t * GPC:(t + 1) * GPC],
            c4[:, t:t + 1],
        )

    # ---- pooling: g[8,64] = sum_t (Mpool*c4)[:,t]^T @ y3[:,t] ----
    g = gps.tile([8, 64], f32, tag="gmlp")
    for t in range(NBLK):
        nc.tensor.matmul(
            g[:],
            mp2[:, t * GPC:(t + 1) * GPC],
            y3[:, t * 64:(t + 1) * 64],
            start=(t == 0), stop=(t == NBLK - 1),
        )

    # ---- MLP head ----
    gsb = spool.tile([8, 64], f32, tag="gsb")
    nc.vector.tensor_copy(gsb[:], g[:])
    gT = gps.tile([64, 8], f32, tag="gmlp")
    nc.tensor.transpose(gT[:], gsb[:], eysb[:])
    gTsb = spool.tile([64, 8], f32r, tag="gTsb")
    nc.vector.tensor_copy(gTsb[:], gT[:])
    hid = gps.tile([32, 8], f32, tag="gmlp")
    nc.tensor.matmul(hid[:], W1sb[:], gTsb[:], start=True, stop=True)
    hsb = spool.tile([32, 8], f32r, tag="hsb")
    nc.scalar.activation(hsb[:], hid[:], AF.Relu, bias=b1sb[:, 0:1], scale=1.0)
    o = gps.tile([1, 8], f32, tag="gmlp")
    nc.tensor.matmul(o[:], W2sb[:], hsb[:], start=True, stop=True)
    osb = spool.tile([1, 8], f32, tag="osb")
    nc.scalar.activation(osb[:], o[:], AF.Identity, bias=b2sb[:, 0:1], scale=1.0)
    nc.sync.dma_start(out, osb[:])


def _prep_inputs(inputs):
    import ml_dtypes

    x = np.ascontiguousarray(np.asarray(inputs["x"], dtype=np.float32))
    batch = np.asarray(inputs["batch"]).astype(np.int64)
    Wn = np.asarray(inputs["Wn"], dtype=np.float32)
    ln_scale = np.asarray(inputs["ln_scale"], dtype=np.float32)
    ln_bias = np.asarray(inputs["ln_bias"], dtype=np.float32)
    W1 = np.asarray(inputs["W1"], dtype=np.float32)
    b1 = np.asarray(inputs["b1"], dtype=np.float32)
    W2 = np.asarray(inputs["W2"], dtype=np.float32)
    b2 = np.asarray(inputs["b2"], dtype=np.float32)
    assert np.allclose(ln_bias, 0.0), "kernel assumes ln_bias == 0"

    C = (np.eye(HID) - np.ones((HID, HID)) / HID).astype(np.float32)
    Bstar = np.eye(HID, dtype=np.float32)
    for l in range(4):
        A = np.eye(HID, dtype=np.float32) + (Wn[l, 0] + Wn[l, 1]) * 0.5
        S = (
            np.diag(ln_scale[l - 1]).astype(np.float32)
            if l > 0 else np.eye(HID, dtype=np.float32)
        )
        Bstar = Bstar @ (S @ A @ C)
    Bstar = np.ascontiguousarray(Bstar.astype(np.float32))
    W1p = np.ascontiguousarray(
        (np.diag(ln_scale[3]).astype(np.float32) @ W1).astype(np.float32)
    )

    bounds = np.searchsorted(batch, np.arange(0, 65, GPC))
    in_maps = []
    for c in range(NCORES):
        s, e = int(bounds[c]), int(bounds[c + 1])
        n = e - s
        assert n <= NPAD, f"core {c} shard {n} > NPAD {NPAD}"
        xTc = np.zeros((64, NPAD), dtype=np.float32)
        xTc[:, :n] = x[s:e].T
        mp = np.zeros((128, NBLK * GPC), dtype=np.float32)
        gb = (batch[s:e] - GPC * c).astype(np.int64)
        idx = np.arange(n)
        mp[idx % 128, (idx // 128) * GPC + gb] = 1.0
        in_maps.append(
            dict(
                xT=xTc,
                Bs=Bstar,
                Mp=np.ascontiguousarray(mp),
                W1=W1p,
                b1=np.ascontiguousarray(b1.reshape(32, 1)),
                W2=np.ascontiguousarray(W2.reshape(32, 1)),
                b2=np.ascontiguousarray(b2.reshape(1, 1)),
                ey=np.eye(8, dtype=np.float32),
            )
        )
    return in_maps


def kernel(**inputs):
    global _prog
    from concourse import bass_utils

    in_maps = _prep_inputs(inputs)
    if _prog is None:
        _prog = _build_program()
    res = bass_utils.run_bass_kernel_spmd(
        _prog, in_maps, core_ids=list(range(NCORES))
    )
    outs = [np.asarray(res.results[c]["out"]).reshape(GPC) for c in range(NCORES)]
    return np.concatenate(outs).reshape(64, 1).astype(np.float32)



# revision 2
# speedup vs baseline: 1.6402x; 1.6402x over previous
"""Trainium2 Bass kernel for nn_MEGANCore (GATv2-style message-passing GNN).

Key insight 1: in the reference, _gatv2 gathers x_j = xp[col] and segment-sums
x_j * alpha by col; softmax weights alpha sum to 1 within each segment (and
self-loops guarantee non-empty segments), so the aggregation is exactly
xp = h @ W: the edges never matter.  The network collapses to a per-node
linear chain + layernorms + pooling + MLP.

Key insight 2 (folding): with ln_bias == 0 (asserted), each layer is
    h_{l+1} = rstd_l * (h_l @ B_l),   B_l = diag(scale_{l-1}) (I + (W0+W1)/2) C
with C = I - 11^T/64 the centering matrix and rstd a per-node scalar.
Per-node scalars commute through the chain; dropping the LN eps=1e-5 inside
the chain the scalars all cancel except a final c4 = 1/sqrt(mean((x@B*)^2))
with B* = B0@B1@B2@B3 precomputed on host.  The device computes only:

    h~ = x @ B*                    (one 64x64 matmul per 128-node block)
    c4 = rsqrt(mean(h~^2, feat))   (per node)
    gT = sum_t y_t^T @ (Mp*c4)_t   (pooling, 8 graphs/core, [64,8])
    out = W2'^T relu(W1'^T [gT;1]) (biases folded via ones-row)

All matmul operands are bf16 (4x faster PE than the fp32 HIGH 4-pass mode,
half the DMA bytes); PSUM accumulation stays fp32, statistics fp32.
Sharding: batch is sorted; 64 graphs -> 8 graphs per core, contiguous node
ranges padded to NPAD.  Host prep is pure data layout + weight folding.

Schedule (per core): x streamed in 4 quarters on the sync HWDGE ring while
consts go on the scalar ring; per quarter: 13 y-matmuls -> ACT square /
DVE evict+reduce -> sqrt/recip -> mp2 on GpSimd -> 13 pooling matmuls,
software-pipelined so pooling of quarter q issues after y of quarter q+1.
"""

import numpy as np

HID = 64
NCORES = 8
GPC = 8                 # graphs per core
NBLK = 52               # 128-node blocks per core
NPAD = NBLK * 128       # 6656 padded nodes per core
NQ = 4                  # x DMA chunks / pipeline stages
QB = NBLK // NQ         # blocks per quarter (13)
EPS_SQ = 1e-9           # guards rsqrt on zero-padded nodes

# const buffer column layout (bf16 [128, CW])
C_BS = 0                # Bs   [64, 64]
C_MP = 64               # Mp   [128, NBLK*GPC]
C_W1 = C_MP + NBLK * GPC    # W1p  [65, 32]  (row 64 = b1)
C_W2 = C_W1 + 32        # W2p  [33, 1]   (row 32 = b2)
C_EPS = C_W2 + 1        # eps  [128, 1]
CW = C_EPS + 1

_prog = None


def _build_program():
    import concourse.tile as tile
    from concourse import bacc, mybir
    from contextlib import ExitStack

    f32 = mybir.dt.float32
    bf16 = mybir.dt.bfloat16

    nc = bacc.Bacc(
        "TRN2", target_bir_lowering=False, debug=False, num_devices=NCORES
    )
    xT = nc.dram_tensor("xT", [64, NPAD], bf16, kind="ExternalInput").ap()
    CS = nc.dram_tensor("CS", [128, CW], bf16, kind="ExternalInput").ap()
    out = nc.dram_tensor("out", [1, GPC], f32, kind="ExternalOutput").ap()

    with tile.TileContext(nc) as tc:
        with ExitStack() as ctx:
            _body(ctx, tc, nc, mybir, xT, CS, out)
    nc.compile()
    return nc


def _body(ctx, tc, nc, mybir, xT, CS, out):
    f32 = mybir.dt.float32
    bf16 = mybir.dt.bfloat16
    AF = mybir.ActivationFunctionType
    AX = mybir.AxisListType
    ALU = mybir.AluOpType

    const = ctx.enter_context(tc.tile_pool(name="const", bufs=1))
    spool = ctx.enter_context(tc.tile_pool(name="scr", bufs=1))
    xpool = ctx.enter_context(tc.tile_pool(name="xp", bufs=1))
    psp = ctx.enter_context(tc.tile_pool(name="psp", bufs=2, space="PSUM"))
    gps = ctx.enter_context(tc.tile_pool(name="gps", bufs=1, space="PSUM"))

    # ---- DMAs: consts on the scalar HWDGE ring, x quarters on sync ----
    csb = const.tile([128, CW], bf16, tag="csb")
    nc.scalar.dma_start(csb[:], CS)
    xsb = xpool.tile([64, NPAD], bf16, tag="xsb")
    QN = QB * 128        # nodes per quarter
    for q in range(NQ):
        nc.sync.dma_start(
            xsb[:, q * QN:(q + 1) * QN], xT[:, q * QN:(q + 1) * QN]
        )

    Bsb = csb[0:64, C_BS:C_BS + 64]
    W1p = csb[0:65, C_W1:C_W1 + 32]
    W2p = csb[0:33, C_W2:C_W2 + 1]
    epsb = csb[:, C_EPS:C_EPS + 1]

    y3 = spool.tile([128, NBLK * 64], bf16, tag="y3")
    sq = spool.tile([128, NBLK * 64], bf16, tag="sq")
    msq = spool.tile([128, NBLK], f32, tag="msq")
    c4a = spool.tile([128, NBLK], f32, tag="c4a")
    c4 = spool.tile([128, NBLK], f32, tag="c4")
    mp2 = spool.tile([128, NBLK * GPC], bf16, tag="mp2")
    gT = gps.tile([64, GPC], f32, tag="gT")

    # ones-rows for folded biases (written once, early)
    gTsb = spool.tile([65, GPC], bf16, tag="gTsb")
    nc.gpsimd.memset(gTsb[64:65, :], 1.0)
    hsb = spool.tile([33, GPC], bf16, tag="hsb")
    nc.gpsimd.memset(hsb[32:33, :], 1.0)

    ps_q = [None] * NQ

    def y_mms(q):
        ps = psp.tile([128, QB * 64], f32, tag="ps")
        ps_q[q] = ps
        for i in range(QB):
            t = q * QB + i
            nc.tensor.matmul(
                ps[:, i * 64:(i + 1) * 64],
                xsb[:, t * 128:(t + 1) * 128],
                Bsb,
                start=True, stop=True,
            )

    def stats(q):
        ps = ps_q[q]
        qf = QB * 64
        # evict h~ (DVE, PSUM 1x) and square (ACT) in parallel
        nc.vector.tensor_copy(y3[:, q * qf:(q + 1) * qf], ps[:])
        nc.scalar.square(sq[:, q * qf:(q + 1) * qf], ps[:])
        nc.vector.tensor_reduce(
            msq[:, q * QB:(q + 1) * QB],
            sq[:, q * qf:(q + 1) * qf].rearrange("p (b f) -> p b f", f=64),
            axis=AX.X, op=ALU.add,
        )
        nc.scalar.activation(
            c4a[:, q * QB:(q + 1) * QB], msq[:, q * QB:(q + 1) * QB],
            AF.Sqrt, bias=epsb, scale=1.0 / 64,
        )
        nc.vector.reciprocal(
            c4[:, q * QB:(q + 1) * QB], c4a[:, q * QB:(q + 1) * QB]
        )
        # mp2 = Mp * c4 broadcast over the 8 graph columns (GpSimd, SBUF-only)
        qg = QB * GPC
        nc.gpsimd.tensor_tensor(
            mp2[:, q * qg:(q + 1) * qg].rearrange("p (b g) -> p b g", g=GPC),
            csb[:, C_MP + q * qg:C_MP + (q + 1) * qg].rearrange(
                "p (b g) -> p b g", g=GPC
            ),
            c4[:, q * QB:(q + 1) * QB].unsqueeze(2).broadcast_to(
                [128, QB, GPC]
            ),
            ALU.mult,
        )

    def pool_mms(q):
        for i in range(QB):
            t = q * QB + i
            nc.tensor.matmul(
                gT[:],
                y3[:, t * 64:(t + 1) * 64],
                mp2[:, t * GPC:(t + 1) * GPC],
                start=(t == 0), stop=(t == NBLK - 1),
            )

    # software pipeline: pool(q) issues on PE after y(q+1)
    y_mms(0)
    stats(0)
    for q in range(1, NQ):
        y_mms(q)
        stats(q)
        pool_mms(q - 1)
    pool_mms(NQ - 1)

    # ---- MLP head: hid = relu(W1'^T [g;1]), out = W2'^T [hid;1] ----
    nc.vector.tensor_copy(gTsb[0:64, :], gT[:])
    hid = gps.tile([32, GPC], f32, tag="hid")
    nc.tensor.matmul(hid[:], W1p, gTsb[0:65, :], start=True, stop=True)
    nc.scalar.activation(hsb[0:32, :], hid[:], AF.Relu, scale=1.0)
    o = gps.tile([1, GPC], f32, tag="o")
    nc.tensor.matmul(o[:], W2p, hsb[0:33, :], start=True, stop=True)
    osb = spool.tile([1, GPC], f32, tag="osb")
    nc.scalar.copy(osb[:], o[:])
    nc.sync.dma_start(out, osb[:])


def _prep_inputs(inputs):
    import ml_dtypes

    bf16 = ml_dtypes.bfloat16
    x = np.ascontiguousarray(np.asarray(inputs["x"], dtype=np.float32))
    batch = np.asarray(inputs["batch"]).astype(np.int64)
    Wn = np.asarray(inputs["Wn"], dtype=np.float32)
    ln_scale = np.asarray(inputs["ln_scale"], dtype=np.float32)
    ln_bias = np.asarray(inputs["ln_bias"], dtype=np.float32)
    W1 = np.asarray(inputs["W1"], dtype=np.float32)
    b1 = np.asarray(inputs["b1"], dtype=np.float32)
    W2 = np.asarray(inputs["W2"], dtype=np.float32)
    b2 = np.asarray(inputs["b2"], dtype=np.float32)
    assert np.allclose(ln_bias, 0.0), "kernel assumes ln_bias == 0"

    C = (np.eye(HID) - np.ones((HID, HID)) / HID).astype(np.float64)
    Bstar = np.eye(HID, dtype=np.float64)
    for l in range(4):
        A = np.eye(HID, dtype=np.float64) + (Wn[l, 0] + Wn[l, 1]) * 0.5
        S = (
            np.diag(ln_scale[l - 1]).astype(np.float64)
            if l > 0 else np.eye(HID, dtype=np.float64)
        )
        Bstar = Bstar @ (S @ A @ C)
    W1p = np.diag(ln_scale[3]).astype(np.float64) @ W1

    bounds = np.searchsorted(batch, np.arange(0, 65, GPC))
    in_maps = []
    for c in range(NCORES):
        s, e = int(bounds[c]), int(bounds[c + 1])
        n = e - s
        assert n <= NPAD, f"core {c} shard {n} > NPAD {NPAD}"
        xTc = np.zeros((64, NPAD), dtype=bf16)
        xTc[:, :n] = x[s:e].T.astype(bf16)
        cs = np.zeros((128, CW), dtype=np.float64)
        cs[0:64, C_BS:C_BS + 64] = Bstar
        gb = (batch[s:e] - GPC * c).astype(np.int64)
        idx = np.arange(n)
        mp = np.zeros((128, NBLK * GPC), dtype=np.float64)
        mp[idx % 128, (idx // 128) * GPC + gb] = 1.0
        cs[:, C_MP:C_MP + NBLK * GPC] = mp
        cs[0:64, C_W1:C_W1 + 32] = W1p
        cs[64, C_W1:C_W1 + 32] = b1
        cs[0:32, C_W2] = W2[:, 0]
        cs[32, C_W2] = b2[0]
        cs[:, C_EPS] = EPS_SQ
        in_maps.append(
            dict(xT=xTc, CS=np.ascontiguousarray(cs.astype(bf16)))
        )
    return in_maps


def kernel(**inputs):
    global _prog
    from concourse import bass_utils

    in_maps = _prep_inputs(inputs)
    if _prog is None:
        _prog = _build_program()
    res = bass_utils.run_bass_kernel_spmd(
        _prog, in_maps, core_ids=list(range(NCORES))
    )
    outs = [np.asarray(res.results[c]["out"]).reshape(GPC) for c in range(NCORES)]
    return np.concatenate(outs).reshape(64, 1).astype(np.float32)
